# revision 4
# baseline (speedup 1.0000x reference)
"""GroupDense kernel for Trainium2 (8 NeuronCores, SPMD data-parallel over batch).

y[b,s,g*64+v] = relu(sum_u x[b,s,g*64+u] * w[g,u,v])
x: [8, 2048, 4096] fp32, w: [64, 64, 64] fp32.

Per-core: core i processes batch i. Host pre-permutes/casts the shard to
x [P=128, CB=32, TOK=2048] bf16 (channel-within-block on partitions) so the
contraction dim lands on SBUF partitions with no on-chip transpose, and
multi-block chunks are DMA-contiguous per partition. Weights are packed into
32 block-diagonal [128,128] bf16 tiles (two 64x64 groups each), loaded in 4
chunks so the first matmul starts early. The matmul runs weight-stationary
(lhsT = w block, rhs = x streaming 512 tokens) producing y^T per block;
ReLU (split ACT/DVE across PSUM banks) writes bf16. Loads ride the SP HWDGE
ring, stores the ACT ring, with the last two stores on SP/SWDGE so all
queues drain the tail. Chunk sizes ramp 1,1,2,4,... so the store stream
starts early (both DMA rings saturate ~420 GB/s aggregate) while the bulk
uses few large DMAs. Host un-permutes y and upcasts to fp32. HBM traffic is
16 MB in + 16 MB out per core.
"""

import numpy as np
import ml_dtypes

import concourse.bass as bass
import concourse.mybir as mybir
import concourse.tile as tile
from concourse import bacc
from concourse.bass import ds, ts
from concourse.bass_utils import run_bass_kernel_spmd

B, S, C = 8, 2048, 4096
U = 64
G = C // U  # 64 groups
NCORES = 8
TOK = (B * S) // NCORES  # 2048 tokens per core
P = 128
CB = C // P  # 32 channel blocks (2 groups each)
NSEG = TOK // 512  # 4 matmul segments of 512 tokens per block
HALF = (NSEG // 2) * 512

# DMA chunking over channel blocks: small at the edges (fast pipeline
# ramp/drain), large in the middle (fewer descriptors, less overhead).
CHUNKS = [1, 1, 2, 4, 4, 4, 4, 4, 4, 2, 1, 1]
assert sum(CHUNKS) == CB

F32 = mybir.dt.float32
BF16 = mybir.dt.bfloat16
BF16NP = ml_dtypes.bfloat16

_cached_nc = None


def _build():
    global _cached_nc
    if _cached_nc is not None:
        return _cached_nc

    nc = bacc.Bacc("TRN2", target_bir_lowering=False)

    x_d = nc.dram_tensor("x", [P, CB, TOK], BF16, kind="ExternalInput")
    w_d = nc.dram_tensor("w2", [P, CB * P], BF16, kind="ExternalInput")
    y_d = nc.dram_tensor("y", [P, CB, TOK], BF16, kind="ExternalOutput")

    with tile.TileContext(nc) as tc:
        with (
            tc.tile_pool(name="wpool", bufs=1) as wpool,
            tc.tile_pool(name="xpool", bufs=3) as xpool,
            tc.tile_pool(name="ypool", bufs=3) as ypool,
            tc.tile_pool(name="ps", bufs=2, space="PSUM") as ps,
        ):
            w_s = wpool.tile([P, CB, P], BF16)
            WBLK = CB // 4
            for c in range(4):
                nc.scalar.dma_start(
                    w_s[:, ds(c * WBLK, WBLK), :],
                    w_d[:, ds(c * WBLK * P, WBLK * P)],
                )

            nchunks = len(CHUNKS)
            c0 = 0
            for ci, J in enumerate(CHUNKS):
                x_t = xpool.tile([P, J, TOK], BF16)
                nc.sync.dma_start(x_t[:], x_d[:, ds(c0, J), :])

                y_t = ypool.tile([P, J, TOK], BF16)
                for j in range(J):
                    cb = c0 + j
                    pY = ps.tile([P, NSEG, 512], F32)
                    for i in range(NSEG):
                        nc.tensor.matmul(
                            pY[:, i, :],
                            w_s[:, cb, :],
                            x_t[:, j, ds(i * 512, 512)],
                            start=True,
                            stop=True,
                        )
                    nc.scalar.activation(
                        y_t[:, j, 0:HALF],
                        pY[:, 0 : NSEG // 2, :],
                        mybir.ActivationFunctionType.Relu,
                    )
                    nc.vector.tensor_scalar_max(
                        y_t[:, j, HALF:TOK], pY[:, NSEG // 2 : NSEG, :], 0.0
                    )
                # last two stores ride otherwise-idle queues so all three
                # DMA queues drain the tail concurrently.
                if ci == nchunks - 2:
                    eng = nc.sync
                elif ci == nchunks - 1:
                    eng = nc.gpsimd
                else:
                    eng = nc.scalar
                eng.dma_start(y_d[:, ds(c0, J), :], y_t[:])
                c0 += J

    nc.compile()
    _cached_nc = nc
    return nc


def _pack_weights(kern):
    # [P, CB*P] bf16: block-diagonal pairs, partition-major (u within block
    # on partitions; blocks x out-channel along the free dim).
    w2 = np.zeros((CB, P, P), dtype=np.float32)
    w2[:, :U, :U] = kern[0::2]
    w2[:, U:, U:] = kern[1::2]
    return np.ascontiguousarray(
        w2.transpose(1, 0, 2).reshape(P, CB * P).astype(BF16NP)
    )


def prep_inputs(x, kern):
    x = np.asarray(x, dtype=np.float32)
    w2 = _pack_weights(np.asarray(kern, dtype=np.float32))
    in_maps = []
    for i in range(NCORES):
        # [TOK, C] -> [P, CB, TOK]: x_h[p, cb, t] = x[t, cb*128 + p]
        xh = x[i].reshape(TOK, CB, P).transpose(2, 1, 0).astype(BF16NP)
        in_maps.append({"x": np.ascontiguousarray(xh), "w2": w2})
    return in_maps


def postprocess(res):
    out = np.empty((NCORES, TOK, C), dtype=np.float32)
    for i in range(NCORES):
        yh = np.asarray(res.results[i]["y"])  # [P, CB, TOK] bf16
        out[i] = yh.transpose(2, 1, 0).reshape(TOK, C).astype(np.float32)
    return np.ascontiguousarray(out.reshape(B, S, C))


def kernel(x, kernel):
    nc = _build()
    in_maps = prep_inputs(x, kernel)
    res = run_bass_kernel_spmd(nc, in_maps, list(range(NCORES)))
    return postprocess(res)


# revision 7
# speedup vs baseline: 1.0040x; 1.0040x over previous
"""GroupDense kernel for Trainium2 (8 NeuronCores, SPMD data-parallel over batch).

y[b,s,g*64+v] = relu(sum_u x[b,s,g*64+u] * w[g,u,v])
x: [8, 2048, 4096] fp32, w: [64, 64, 64] fp32.

Per-core: core i processes batch i. Host pre-permutes/casts the shard to
x [P=128, CB=32, TOK=2048] bf16 (channel-within-block on partitions) so the
contraction dim lands on SBUF partitions with no on-chip transpose, and
multi-block chunks are DMA-contiguous per partition. Weights are packed into
32 block-diagonal [128,128] bf16 tiles (two 64x64 groups each), loaded in 4
chunks so the first matmul starts early. The matmul runs weight-stationary
(lhsT = w block, rhs = x streaming 512 tokens) producing y^T per block;
ReLU (split ACT/DVE across PSUM banks) writes bf16. Loads ride the SP HWDGE
ring, stores the ACT ring, with the last two stores on SP/SWDGE so all
queues drain the tail. Chunk sizes ramp 1,1,2,4,... so the store stream
starts early (both DMA rings saturate ~420 GB/s aggregate) while the bulk
uses few large DMAs. Host un-permutes y and upcasts to fp32. HBM traffic is
16 MB in + 16 MB out per core.
"""

import numpy as np
import ml_dtypes

import concourse.bass as bass
import concourse.mybir as mybir
import concourse.tile as tile
from concourse import bacc
from concourse.bass import ds, ts
from concourse.bass_utils import run_bass_kernel_spmd

B, S, C = 8, 2048, 4096
U = 64
G = C // U  # 64 groups
NCORES = 8
TOK = (B * S) // NCORES  # 2048 tokens per core
P = 128
CB = C // P  # 32 channel blocks (2 groups each)
NSEG = TOK // 512  # 4 matmul segments of 512 tokens per block
HALF = (NSEG // 2) * 512

# DMA chunking over channel blocks: small at the edges (fast pipeline
# ramp/drain), 2-block chunks in the middle (8 KB contiguous per-partition
# descriptors without making the pipeline lumpy).
CHUNKS = [1, 1, 1] + [2] * 13 + [1, 1, 1]
assert sum(CHUNKS) == CB

F32 = mybir.dt.float32
BF16 = mybir.dt.bfloat16
BF16NP = ml_dtypes.bfloat16

_cached_nc = None


def _build():
    global _cached_nc
    if _cached_nc is not None:
        return _cached_nc

    nc = bacc.Bacc("TRN2", target_bir_lowering=False)

    x_d = nc.dram_tensor("x", [P, CB, TOK], BF16, kind="ExternalInput")
    w_d = nc.dram_tensor("w2", [P, CB * P], BF16, kind="ExternalInput")
    y_d = nc.dram_tensor("y", [P, CB, TOK], BF16, kind="ExternalOutput")

    with tile.TileContext(nc) as tc:
        with (
            tc.tile_pool(name="wpool", bufs=1) as wpool,
            tc.tile_pool(name="xpool", bufs=4) as xpool,
            tc.tile_pool(name="ypool", bufs=4) as ypool,
            tc.tile_pool(name="ps", bufs=2, space="PSUM") as ps,
        ):
            w_s = wpool.tile([P, CB, P], BF16)
            WBLK = CB // 4
            for c in range(4):
                nc.scalar.dma_start(
                    w_s[:, ds(c * WBLK, WBLK), :],
                    w_d[:, ds(c * WBLK * P, WBLK * P)],
                )

            nchunks = len(CHUNKS)
            c0 = 0
            for ci, J in enumerate(CHUNKS):
                x_t = xpool.tile([P, J, TOK], BF16)
                nc.sync.dma_start(x_t[:], x_d[:, ds(c0, J), :])

                y_t = ypool.tile([P, J, TOK], BF16)
                for j in range(J):
                    cb = c0 + j
                    pY = ps.tile([P, NSEG, 512], F32)
                    for i in range(NSEG):
                        nc.tensor.matmul(
                            pY[:, i, :],
                            w_s[:, cb, :],
                            x_t[:, j, ds(i * 512, 512)],
                            start=True,
                            stop=True,
                        )
                    nc.scalar.activation(
                        y_t[:, j, 0:HALF],
                        pY[:, 0 : NSEG // 2, :],
                        mybir.ActivationFunctionType.Relu,
                    )
                    nc.vector.tensor_scalar_max(
                        y_t[:, j, HALF:TOK], pY[:, NSEG // 2 : NSEG, :], 0.0
                    )
                # tail: alternate the last stores across both HWDGE rings so
                # they drain concurrently once loads are done.
                if ci in (nchunks - 3, nchunks - 1):
                    eng = nc.sync
                else:
                    eng = nc.scalar
                eng.dma_start(y_d[:, ds(c0, J), :], y_t[:])
                c0 += J

    nc.compile()
    _cached_nc = nc
    return nc


def _pack_weights(kern):
    # [P, CB*P] bf16: block-diagonal pairs, partition-major (u within block
    # on partitions; blocks x out-channel along the free dim).
    w2 = np.zeros((CB, P, P), dtype=np.float32)
    w2[:, :U, :U] = kern[0::2]
    w2[:, U:, U:] = kern[1::2]
    return np.ascontiguousarray(
        w2.transpose(1, 0, 2).reshape(P, CB * P).astype(BF16NP)
    )


def prep_inputs(x, kern):
    x = np.asarray(x, dtype=np.float32)
    w2 = _pack_weights(np.asarray(kern, dtype=np.float32))
    in_maps = []
    for i in range(NCORES):
        # [TOK, C] -> [P, CB, TOK]: x_h[p, cb, t] = x[t, cb*128 + p]
        xh = x[i].reshape(TOK, CB, P).transpose(2, 1, 0).astype(BF16NP)
        in_maps.append({"x": np.ascontiguousarray(xh), "w2": w2})
    return in_maps


def postprocess(res):
    out = np.empty((NCORES, TOK, C), dtype=np.float32)
    for i in range(NCORES):
        yh = np.asarray(res.results[i]["y"])  # [P, CB, TOK] bf16
        out[i] = yh.transpose(2, 1, 0).reshape(TOK, C).astype(np.float32)
    return np.ascontiguousarray(out.reshape(B, S, C))


def kernel(x, kernel):
    nc = _build()
    in_maps = prep_inputs(x, kernel)
    res = run_bass_kernel_spmd(nc, in_maps, list(range(NCORES)))
    return postprocess(res)
